# revision 11
# baseline (speedup 1.0000x reference)
"""Trainium2 Bass kernel for the DEER-MLP spiking network (v4).

Network: x(4,32,196,384) -> FC1(384->1536) -> BatchNorm -> LIF(T=4) ->
FC2(1536->384) -> BatchNorm -> LIF -> spikes(4,32,196,384).

Math notes:
 - The reference's 10 DEER Newton iterations over T=4 steps converge to the
   exact sequential LIF recurrence; we compute that directly.
 - The pre-BN biases b1/b2 cancel inside BatchNorm (additive per-channel
   constants are removed by the mean subtraction), so they are dropped.
 - Both matmuls run as multi-pass fp16 with hi/lo fp16 limbs (exact to
   ~2^-22): FC1 = x_hi@w_hi + x_lo@w_hi + x_hi@w_lo; FC2 spikes are exact
   in fp16 so two passes (w_hi + w_lo) suffice.

Distribution: data-parallel over B across 8 cores (784 lanes/core).
BatchNorm statistics are the only cross-core coupling: a warmup dummy
AllReduce (absorbs the runtime's ~50us CC-stream init barrier), four
staged BN1 stat AllReduces (pipelined under FC1), and one BN2 AllReduce.

v4 schedule / queue discipline:
 - FC1 h-tile-outer; y1 spills to DRAM through a bounce ring so FC1 never
   stalls on BN1-stats latency.  Everything stats-critical rides the
   (never-gated) scalar queue: the per-chunk sum via accum_out on the
   Identity evacuation, the sumsq via a Square written back into the dead
   PSUM bank, and the group reductions via tiny accum reads.  The sync
   queue carries only input loads / spills / outputs.
 - All AR-gated work (BN1 affine, LIF1, coeffs, y1 reloads) rides the DVE
   queue, which pays the AR latency anyway; reload DMAs issue there so
   they can never head-of-line block the spill path.
 - The last stats group (h-tiles 9-11) is processed in FC2-m-tile-sized
   slice reloads interleaved with FC2 group emission, so FC2 starts right
   after the final stats AllReduce and never waits again.
 - FC2 uses s1 as the stationary operand, producing y2 ROW-major; w2 is
   prefetched at kernel start into its own pool (no WAR on freed space).
   BN2 stats: DVE accumulate + ones-matmul partition reduce; coeffs
   broadcast to 128 partitions with a rank-1 matmul.  LIF2 emits spikes
   row-major and DMAs straight out - no transpose phase.
"""

import numpy as np

import concourse.bass as bass
import concourse.mybir as mybir
import concourse.tile as tile
from concourse import bacc
from concourse.bass_utils import run_bass_kernel_spmd

F32 = mybir.dt.float32
F16 = mybir.dt.float16
AF = mybir.ActivationFunctionType
OP = mybir.AluOpType
AX = mybir.AxisListType

T, B, NN, C, H = 4, 32, 196, 384, 1536
NCORES = 8
BLOC = B // NCORES            # 4 batches per core
MLOC = BLOC * NN              # 784 lanes per core
R = T * MLOC                  # 3136 flattened (t, m) rows per core
NTOT = T * B * NN             # 25088 batchnorm samples per channel
KC = C // 128                 # 3 c-tiles
KH = H // 128                 # 12 h-tiles
EPS = 1e-5
P = 128

CH = 448                      # FC1 moving-operand chunk (7 * 448 = 3136)
NCH = R // CH
MT = 128                      # FC2 m-tile (rows per output group)
NMT = MLOC // MT              # 6 full m-tiles per t step
MTAIL = MLOC - NMT * MT       # 16 tail rows per t step
HGRP = 3                      # h-tiles per BN1 stats allreduce group
NGRP = KH // HGRP             # 4 staged allreduces
LSL = MLOC // 2               # LIF1 slice width (full tiles)


def _lif1_steps(nc, pool, yv, s1a, ms, md, mlen, tg, nm):
    """LIF chain reading drive slices yv[:, t, ms:ms+mlen] (already
    affined); spikes into s1a[:, t, md:md+mlen] fp16."""
    v = None
    for t in range(T):
        d = yv[:, t, ms : ms + mlen]
        if v is None:
            h = d
        else:
            ht = pool.tile([P, mlen], F32, tag=f"{tg}_h", name=f"{nm}h{t}")
            nc.vector.scalar_tensor_tensor(ht[:], v, 0.5, d, OP.mult, OP.add)
            h = ht[:]
        nc.vector.tensor_scalar(s1a[:, t, md : md + mlen], h, 1.0, None,
                                OP.is_ge)
        if t < T - 1:
            vt = pool.tile([P, mlen], F32, tag=f"{tg}_v", name=f"{nm}v{t}")
            nc.vector.scalar_tensor_tensor(vt[:], h, 1.0, h, OP.is_lt,
                                           OP.mult)
            v = vt[:]


def _bn_coeffs(nc, pool, stg, gt2, bet2, k, pp, tag, dsc_out, dsh_out):
    """From allreduced [pp, 2k] (sum || sumsq) write the fused affine
    coeffs: drive = y*dsc + dsh == 0.5*((y - mean)*rsqrt(var+eps)*g + be).
    gt2/bet2 must be pre-scaled by 0.5."""
    mean = pool.tile([pp, k], F32, tag=f"{tag}_mean", name=f"{tag}_mean")
    nc.vector.tensor_scalar(mean[:], stg[:, 0:k], 1.0 / NTOT, None, OP.mult)
    msq = pool.tile([pp, k], F32, tag=f"{tag}_msq", name=f"{tag}_msq")
    nc.vector.tensor_tensor(msq[:], mean[:], mean[:], OP.mult)
    var = pool.tile([pp, k], F32, tag=f"{tag}_var", name=f"{tag}_var")
    nc.vector.scalar_tensor_tensor(var[:], stg[:, k : 2 * k], 1.0 / NTOT,
                                   msq[:], OP.mult, OP.subtract)
    nc.vector.tensor_scalar(var[:], var[:], EPS, None, OP.add)
    std = pool.tile([pp, k], F32, tag=f"{tag}_std", name=f"{tag}_std")
    nc.scalar.activation(std[:], var[:], AF.Sqrt, bias=0.0, scale=1.0)
    rstd = pool.tile([pp, k], F32, tag=f"{tag}_rstd", name=f"{tag}_rstd")
    nc.vector.reciprocal(rstd[:], std[:])
    nc.vector.tensor_tensor(dsc_out, rstd[:], gt2, OP.mult)
    nc.vector.tensor_tensor(dsh_out, mean[:], dsc_out, OP.mult)
    nc.vector.tensor_tensor(dsh_out, bet2, dsh_out, OP.subtract)


def _build():
    nc = bacc.Bacc("TRN2", target_bir_lowering=False, debug=False,
                   num_devices=NCORES)

    xh_d = nc.dram_tensor("xthi", [KC, P, R], F16, kind="ExternalInput")
    xl_d = nc.dram_tensor("xtlo", [KC, P, R], F16, kind="ExternalInput")
    w1h_d = nc.dram_tensor("w1thi", [KC, P, H], F16, kind="ExternalInput")
    w1l_d = nc.dram_tensor("w1tlo", [KC, P, H], F16, kind="ExternalInput")
    w2h_d = nc.dram_tensor("w2thi", [KH, P, C], F16, kind="ExternalInput")
    w2l_d = nc.dram_tensor("w2tlo", [KH, P, C], F16, kind="ExternalInput")
    g1_d = nc.dram_tensor("g1", [H], F32, kind="ExternalInput")
    be1_d = nc.dram_tensor("be1", [H], F32, kind="ExternalInput")
    g2_d = nc.dram_tensor("g2", [C], F32, kind="ExternalInput")
    be2_d = nc.dram_tensor("be2", [C], F32, kind="ExternalInput")
    out_d = nc.dram_tensor("out", [R, C], F32, kind="ExternalOutput")

    groups = [list(range(NCORES))]

    with tile.TileContext(nc) as tc:
        with (
            tc.tile_pool(name="const", bufs=1) as const,
            tc.tile_pool(name="dram", bufs=1, space="DRAM") as dram,
            tc.tile_pool(name="pw2", bufs=1) as pw2,
        ):
            def colvec(dst_k, src, half=False):
                t_ = const.tile([P, dst_k], F32, name=f"cv_{src.name}",
                                tag=f"cv_{src.name}")
                nc.sync.dma_start(
                    t_[:], src.ap().rearrange("(a p) -> p a", p=P))
                if half:
                    nc.vector.tensor_scalar(t_[:], t_[:], 0.5, None, OP.mult)
                return t_

            def rowvec(src, n, half=False):
                t_ = const.tile([1, n], F32, name=f"rv_{src.name}",
                                tag=f"rv_{src.name}")
                nc.sync.dma_start(t_[:], src.ap().rearrange("(a c) -> a c",
                                                            a=1))
                if half:
                    nc.vector.tensor_scalar(t_[:], t_[:], 0.5, None, OP.mult)
                return t_

            g1t, be1t = colvec(KH, g1_d, True), colvec(KH, be1_d, True)
            g2r, be2r = rowvec(g2_d, C, True), rowvec(be2_d, C, True)

            onesc = const.tile([P, 1], F32)
            nc.vector.memset(onesc[:], 1.0)
            ones1r = const.tile([1, P], F32)
            nc.vector.memset(ones1r[:], 1.0)

            # Warmup collective: absorbs the CC-stream init barrier +
            # launch skew in the shadow of the input DMAs.  Consumed
            # (times zero) in phase B so it can't be DCE'd.
            bar_s = const.tile([1, 8], F32)
            nc.vector.memset(bar_s[:], 1.0)
            bar_in = dram.tile([1, 8], F32, tag="bar_in", name="bar_in")
            bar_out = dram.tile([1, 8], F32, tag="bar_out", name="bar_out")
            nc.gpsimd.dma_start(bar_in[:], bar_s[:])
            nc.gpsimd.collective_compute(
                "AllReduce", OP.add, replica_groups=groups,
                ins=[bar_in.opt()], outs=[bar_out.opt()])

            # persistent across phases
            s1 = [const.tile([P, T, MLOC], F16, tag=f"s1_{k}",
                             name=f"s1_{k}") for k in range(KH)]
            asum1 = const.tile([P, KH * NCH], F32)
            asq1 = const.tile([P, KH * NCH], F32)
            junkA = const.tile([P, NCH], F32)
            dsc1 = const.tile([P, KH], F32)
            dsh1 = const.tile([P, KH], F32)

            y1d = [dram.tile([P, R], F32, tag=f"y1d{a}", name=f"y1d{a}")
                   for a in range(KH)]
            st_in = [dram.tile([P, 2 * HGRP], F32, tag=f"sti{g}",
                               name=f"sti{g}") for g in range(NGRP)]
            st_out = [dram.tile([P, 2 * HGRP], F32, tag=f"sto{g}",
                                name=f"sto{g}") for g in range(NGRP)]
            st2_in = dram.tile([1, 2 * C], F32)
            st2_out = dram.tile([1, 2 * C], F32)

            # ---- phase A: FC1 (h-outer) + staged BN1 stats + LIF1 ------
            with (
                tc.tile_pool(name="pax", bufs=1) as pax,
                tc.tile_pool(name="pbn", bufs=3) as pbn,
                tc.tile_pool(name="prel", bufs=3) as prel,
                tc.tile_pool(name="plif", bufs=2) as plif,
                tc.tile_pool(name="ps_mm", bufs=1, space="PSUM") as ps_mm,
            ):
                w1h = pax.tile([P, KC, H], F16)
                nc.sync.dma_start(w1h[:],
                                  w1h_d.ap().rearrange("k p h -> p k h"))
                xh = pax.tile([P, KC, R], F16)
                nc.sync.dma_start(xh[:], xh_d.ap().rearrange("k p r -> p k r"))
                w1l = pax.tile([P, KC, H], F16)
                nc.sync.dma_start(w1l[:],
                                  w1l_d.ap().rearrange("k p h -> p k h"))
                xl = pax.tile([P, KC, R], F16)
                nc.sync.dma_start(xl[:], xl_d.ap().rearrange("k p r -> p k r"))
                w2h = pw2.tile([P, KH, C], F16)
                nc.sync.dma_start(w2h[:],
                                  w2h_d.ap().rearrange("k p c -> p k c"))
                w2l = pw2.tile([P, KH, C], F16)
                nc.sync.dma_start(w2l[:],
                                  w2l_d.ap().rearrange("k p c -> p k c"))

                rel = [None] * KH

                def ensure_rel(a_):
                    # reloads ride the DVE queue: paced with the (equally
                    # AR-gated) LIF work, never blocking spills
                    if rel[a_] is None:
                        rt = prel.tile([P, R], F32, tag="rel",
                                       name=f"rel{a_}")
                        nc.gpsimd.dma_start(rt[:], y1d[a_][:])
                        rel[a_] = rt

                def full_lif(a_):
                    ensure_rel(a_)
                    yv = rel[a_][:].rearrange("p (t m) -> p t m", t=T)
                    nc.vector.tensor_scalar(
                        yv[:, :, 0:MLOC], yv[:, :, 0:MLOC],
                        dsc1[:, a_ : a_ + 1],
                        dsh1[:, a_ : a_ + 1], OP.mult, OP.add)
                    for m0 in range(0, MLOC, LSL):
                        _lif1_steps(nc, plif, yv, s1[a_], m0, m0, LSL,
                                    "l1", f"l1_{a_}_{m0}")

                lif_pending = []
                for a in range(KH):
                    pss = [ps_mm.tile([P, CH], F32, tag=f"mm{c}",
                                      name=f"ps{a}_{c}") for c in range(NCH)]
                    idx = 0
                    for wt, xt in ((w1h, xh), (w1l, xh), (w1h, xl)):
                        for k in range(KC):
                            for c in range(NCH):
                                nc.tensor.matmul(
                                    pss[c][:],
                                    wt[:, k, a * P : (a + 1) * P],
                                    xt[:, k, c * CH : (c + 1) * CH],
                                    start=(idx == 0),
                                    stop=(idx == 8),
                                )
                            idx += 1
                    for c in range(NCH):
                        # evacuate through a bounce ring to DRAM; per-chunk
                        # sum rides the evac via accum_out, sumsq via a
                        # Square written back into the dead psum bank
                        bt = pbn.tile([P, CH], F32, tag="bn",
                                      name=f"bn{a}_{c}")
                        nc.scalar.activation(bt[:], pss[c][:], AF.Identity,
                                             bias=0.0, scale=1.0,
                                             accum_out=asum1[:, a * NCH + c :
                                                             a * NCH + c + 1])
                        nc.scalar.activation(
                            pss[c][:], pss[c][:], AF.Square,
                            bias=0.0, scale=1.0,
                            accum_out=asq1[:, a * NCH + c :
                                           a * NCH + c + 1])
                        nc.sync.dma_start(y1d[a][:, c * CH : (c + 1) * CH],
                                          bt[:])

                    if a % HGRP == HGRP - 1:
                        g = a // HGRP
                        a0 = g * HGRP
                        stg = const.tile([P, 2 * HGRP], F32,
                                         tag=f"stg{g}", name=f"stg{g}")
                        # group-reduce over chunks on the scalar queue
                        # (tiny accum reads) so stat triggers never wait
                        # behind AR-gated DVE work
                        for i in range(HGRP):
                            c0 = (a0 + i) * NCH
                            nc.scalar.activation(
                                junkA[:], asum1[:, c0 : c0 + NCH],
                                AF.Identity, bias=0.0, scale=1.0,
                                accum_out=stg[:, i : i + 1])
                            nc.scalar.activation(
                                junkA[:], asq1[:, c0 : c0 + NCH],
                                AF.Identity, bias=0.0, scale=1.0,
                                accum_out=stg[:, HGRP + i : HGRP + i + 1])
                        nc.gpsimd.dma_start(st_in[g][:], stg[:])
                        nc.gpsimd.collective_compute(
                            "AllReduce", OP.add, replica_groups=groups,
                            ins=[st_in[g].opt()], outs=[st_out[g].opt()])
                        stga = const.tile([P, 2 * HGRP], F32,
                                          tag=f"stga{g}", name=f"stga{g}")
                        nc.gpsimd.dma_start(stga[:], st_out[g][:])
                        _bn_coeffs(nc, const, stga, g1t[:, a0 : a0 + HGRP],
                                   be1t[:, a0 : a0 + HGRP], HGRP, P,
                                   f"bn1g{g}",
                                   dsc1[:, a0 : a0 + HGRP],
                                   dsh1[:, a0 : a0 + HGRP])
                        if g < NGRP - 1:
                            lif_pending.extend(range(a0, a0 + HGRP))

                    # drain one pending LIF h-tile per step (one stats
                    # group behind, so the DVE work hides under FC1)
                    if a >= HGRP and lif_pending:
                        ap_ = lif_pending.pop(0)
                        ensure_rel(ap_)
                        if lif_pending:
                            ensure_rel(lif_pending[0])
                        full_lif(ap_)

                while lif_pending:
                    full_lif(lif_pending.pop(0))

            # ---- phase B: last-group LIF (slice reloads) + FC2 ---------
            with (
                tc.tile_pool(name="pbc", bufs=1) as pbc,
                tc.tile_pool(name="pb", bufs=3) as pb,
                tc.tile_pool(name="prelB", bufs=4) as prelB,
                tc.tile_pool(name="plifB", bufs=2) as plifB,
                tc.tile_pool(name="ps2", bufs=5, space="PSUM") as ps2p,
                tc.tile_pool(name="ps_st", bufs=1, space="PSUM") as ps_st,
            ):
                acc_s = pbc.tile([P, C], F32)
                acc_q = pbc.tile([P, C], F32)
                nc.vector.memset(acc_s[:], 0.0)
                nc.vector.memset(acc_q[:], 0.0)

                a0 = (NGRP - 1) * HGRP    # h-tiles 9..11: sliced here
                y2 = {}
                s1f = [s1[k][:].rearrange("p t m -> p (t m)")
                       for k in range(KH)]
                for jb in range(NMT + 1):
                    m0 = jb * MT
                    mlen = MT if jb < NMT else MTAIL
                    # finish s1 for the last stats group on this m-slice,
                    # then immediately emit the FC2 groups that need it
                    for a_ in range(a0, KH):
                        rsl = prelB.tile([P, T, MT], F32, tag="rb",
                                         name=f"rb{a_}_{jb}")
                        src = y1d[a_][:].rearrange("p (t m) -> p t m", t=T)
                        nc.gpsimd.dma_start(rsl[:, :, 0:mlen],
                                            src[:, :, m0 : m0 + mlen])
                        nc.vector.tensor_scalar(
                            rsl[:, :, 0:mlen], rsl[:, :, 0:mlen],
                            dsc1[:, a_ : a_ + 1], dsh1[:, a_ : a_ + 1],
                            OP.mult, OP.add)
                        _lif1_steps(nc, plifB, rsl, s1[a_], 0, m0, mlen,
                                    "lb" if mlen == MT else "lbt",
                                    f"lb{a_}_{jb}")
                    for t in range(T):
                        ps2 = ps2p.tile([P, C], F32, tag="mm2",
                                        name=f"ps2_{jb}_{t}")
                        idx = 0
                        for k in range(KH):
                            for wsp in (w2h, w2l):
                                nc.tensor.matmul(
                                    ps2[:mlen, :],
                                    s1f[k][:, t * MLOC + m0 :
                                           t * MLOC + m0 + mlen],
                                    wsp[:, k, :],
                                    start=(idx == 0),
                                    stop=(idx == 2 * KH - 1),
                                )
                                idx += 1
                        y2t = pbc.tile([P, C], F32, tag=f"y2_{jb}_{t}",
                                       name=f"y2_{jb}_{t}")
                        y2[(jb, t)] = y2t
                        nc.scalar.activation(y2t[:mlen, :], ps2[:mlen, :],
                                             AF.Identity, bias=0.0,
                                             scale=1.0)
                        sq2 = pb.tile([P, C], F32, tag="sq2")
                        nc.scalar.activation(sq2[:mlen, :], ps2[:mlen, :],
                                             AF.Square, bias=0.0, scale=1.0)
                        nc.vector.tensor_tensor(acc_s[:mlen, :],
                                                acc_s[:mlen, :],
                                                y2t[:mlen, :], OP.add)
                        nc.vector.tensor_tensor(acc_q[:mlen, :],
                                                acc_q[:mlen, :],
                                                sq2[:mlen, :], OP.add)

                # BN2 stats: partition-reduce via ones-matmul, allreduce
                stp_s = ps_st.tile([1, C], F32, tag="sts")
                nc.tensor.matmul(stp_s[:], onesc[:], acc_s[:],
                                 start=True, stop=True)
                stp_q = ps_st.tile([1, C], F32, tag="stq")
                nc.tensor.matmul(stp_q[:], onesc[:], acc_q[:],
                                 start=True, stop=True)
                # consume the warmup-barrier output here (cheap, off the
                # critical path, keeps it from being dead-code-eliminated)
                bar_r = pbc.tile([1, 8], F32)
                nc.gpsimd.dma_start(bar_r[:], bar_out[:])
                st2 = pbc.tile([1, 2 * C], F32)
                nc.vector.tensor_copy(st2[:, 0:C], stp_s[:])
                nc.vector.tensor_copy(st2[:, C : 2 * C], stp_q[:])
                nc.vector.scalar_tensor_tensor(
                    st2[0:1, 0:8], bar_r[:], 0.0, st2[0:1, 0:8],
                    OP.mult, OP.add)
                nc.gpsimd.dma_start(st2_in[:], st2[:])
                nc.gpsimd.collective_compute(
                    "AllReduce", OP.add, replica_groups=groups,
                    ins=[st2_in.opt()], outs=[st2_out.opt()])
                stg2 = pbc.tile([1, 2 * C], F32)
                nc.gpsimd.dma_start(stg2[:], st2_out[:])
                dsc2 = pbc.tile([1, C], F32)
                dsh2 = pbc.tile([1, C], F32)
                _bn_coeffs(nc, pbc, stg2, g2r[:], be2r[:], C, 1, "bn2",
                           dsc2[:], dsh2[:])

                # broadcast [1, C] coeffs to all 128 partitions via PE
                dscB = pbc.tile([P, C], F32)
                dshB = pbc.tile([P, C], F32)
                for src, dst in ((dsc2, dscB), (dsh2, dshB)):
                    bps = ps_st.tile([P, C], F32, tag="bc")
                    nc.tensor.matmul(bps[:], ones1r[:], src[:],
                                     start=True, stop=True)
                    nc.vector.tensor_copy(dst[:], bps[:])

                # ---- phase C: BN2 affine + LIF2 + direct row-major out --
                for jb in range(NMT + 1):
                    mlen = MT if jb < NMT else MTAIL
                    v = None
                    for t in range(T):
                        y2t = y2[(jb, t)]
                        nc.vector.tensor_tensor(
                            y2t[:mlen, :], y2t[:mlen, :], dscB[:mlen, :],
                            OP.mult)
                        nc.vector.tensor_tensor(
                            y2t[:mlen, :], y2t[:mlen, :], dshB[:mlen, :],
                            OP.add)
                        d = y2t[:mlen, :]
                        if v is None:
                            h = d
                        else:
                            ht = pb.tile([P, C], F32, tag="l2_h",
                                         name=f"l2h{jb}_{t}")
                            nc.vector.scalar_tensor_tensor(
                                ht[:mlen, :], v, 0.5, d, OP.mult, OP.add)
                            h = ht[:mlen, :]
                        ob = pb.tile([P, C], F32, tag="ob",
                                     name=f"ob{jb}_{t}")
                        nc.vector.tensor_scalar(ob[:mlen, :], h, 1.0, None,
                                                OP.is_ge)
                        if t < T - 1:
                            vt = pb.tile([P, C], F32, tag="l2_v",
                                         name=f"l2v{jb}_{t}")
                            nc.vector.scalar_tensor_tensor(
                                vt[:mlen, :], h, 1.0, h, OP.is_lt, OP.mult)
                            v = vt[:mlen, :]
                        r0 = t * MLOC + jb * MT
                        nc.sync.dma_start(out_d[r0 : r0 + mlen, :],
                                          ob[:mlen, :])

    nc.compile()
    return nc


_NC = None
TRACE = False          # set by test harness to capture an NTFF profile
LAST_RESULT = None     # BassKernelResults of the most recent run


def _get_nc():
    global _NC
    if _NC is None:
        _NC = _build()
    return _NC


def _split_f16(a):
    hi = a.astype(np.float16)
    lo = (a - hi.astype(np.float32)).astype(np.float16)
    return np.ascontiguousarray(hi), np.ascontiguousarray(lo)


def _in_maps(x, W1, b1, g1, be1, W2, b2, g2, be2):
    x = np.asarray(x, dtype=np.float32)
    w1t = np.asarray(W1, np.float32).T.reshape(KC, P, H)
    w1thi, w1tlo = _split_f16(w1t)
    w2t = np.asarray(W2, np.float32).T.reshape(KH, P, C)
    w2thi, w2tlo = _split_f16(w2t)
    shared = {
        "w1thi": w1thi, "w1tlo": w1tlo,
        "w2thi": w2thi, "w2tlo": w2tlo,
        "g1": np.asarray(g1, np.float32),
        "be1": np.asarray(be1, np.float32),
        "g2": np.asarray(g2, np.float32),
        "be2": np.asarray(be2, np.float32),
    }
    in_maps = []
    for i in range(NCORES):
        xt = x[:, i * BLOC : (i + 1) * BLOC].reshape(R, C).T.reshape(KC, P, R)
        xthi, xtlo = _split_f16(xt)
        in_maps.append({"xthi": xthi, "xtlo": xtlo, **shared})
    return in_maps


def kernel(x, W1, b1, g1, be1, W2, b2, g2, be2):
    nc = _get_nc()
    in_maps = _in_maps(x, W1, b1, g1, be1, W2, b2, g2, be2)
    res = run_bass_kernel_spmd(nc, in_maps, core_ids=list(range(NCORES)),
                               trace=TRACE)
    global LAST_RESULT
    LAST_RESULT = res
    out = np.concatenate(
        [res.results[i]["out"].reshape(T, BLOC, NN, C) for i in range(NCORES)],
        axis=1,
    )
    return out


# revision 12
# speedup vs baseline: 1.1367x; 1.1367x over previous
"""Trainium2 Bass kernel for the DEER-MLP spiking network (v4).

Network: x(4,32,196,384) -> FC1(384->1536) -> BatchNorm -> LIF(T=4) ->
FC2(1536->384) -> BatchNorm -> LIF -> spikes(4,32,196,384).

Math notes:
 - The reference's 10 DEER Newton iterations over T=4 steps converge to the
   exact sequential LIF recurrence; we compute that directly.
 - The pre-BN biases b1/b2 cancel inside BatchNorm (additive per-channel
   constants are removed by the mean subtraction), so they are dropped.
 - Both matmuls run as multi-pass fp16 with hi/lo fp16 limbs (exact to
   ~2^-22): FC1 = x_hi@w_hi + x_lo@w_hi + x_hi@w_lo; FC2 spikes are exact
   in fp16 so two passes (w_hi + w_lo) suffice.

Distribution: data-parallel over B across 8 cores (784 lanes/core).
BatchNorm statistics are the only cross-core coupling: a warmup dummy
AllReduce (absorbs the runtime's ~50us CC-stream init barrier), four
staged BN1 stat AllReduces (pipelined under FC1), and one BN2 AllReduce.

v4 schedule / queue discipline:
 - FC1 h-tile-outer; y1 spills to DRAM through a bounce ring so FC1 never
   stalls on BN1-stats latency.  Everything stats-critical rides the
   (never-gated) scalar queue: the per-chunk sum via accum_out on the
   Identity evacuation, the sumsq via a Square written back into the dead
   PSUM bank, and the group reductions via tiny accum reads.  The sync
   queue carries only input loads / spills / outputs.
 - All AR-gated work (BN1 affine, LIF1, coeffs, y1 reloads) rides the DVE
   queue, which pays the AR latency anyway; reload DMAs issue there so
   they can never head-of-line block the spill path.
 - The last stats group (h-tiles 9-11) is processed in FC2-m-tile-sized
   slice reloads interleaved with FC2 group emission, so FC2 starts right
   after the final stats AllReduce and never waits again.
 - FC2 uses s1 as the stationary operand, producing y2 ROW-major; w2 is
   prefetched at kernel start into its own pool (no WAR on freed space).
   BN2 stats: DVE accumulate + ones-matmul partition reduce; coeffs
   broadcast to 128 partitions with a rank-1 matmul.  LIF2 emits spikes
   row-major and DMAs straight out - no transpose phase.
"""

import numpy as np

import concourse.bass as bass
import concourse.mybir as mybir
import concourse.tile as tile
from concourse import bacc
from concourse.bass_utils import run_bass_kernel_spmd

F32 = mybir.dt.float32
F16 = mybir.dt.float16
AF = mybir.ActivationFunctionType
OP = mybir.AluOpType
AX = mybir.AxisListType

T, B, NN, C, H = 4, 32, 196, 384, 1536
NCORES = 8
BLOC = B // NCORES            # 4 batches per core
MLOC = BLOC * NN              # 784 lanes per core
R = T * MLOC                  # 3136 flattened (t, m) rows per core
NTOT = T * B * NN             # 25088 batchnorm samples per channel
KC = C // 128                 # 3 c-tiles
KH = H // 128                 # 12 h-tiles
EPS = 1e-5
P = 128

CH = 448                      # FC1 moving-operand chunk (7 * 448 = 3136)
NCH = R // CH
MT = 128                      # FC2 m-tile (rows per output group)
NMT = MLOC // MT              # 6 full m-tiles per t step
MTAIL = MLOC - NMT * MT       # 16 tail rows per t step
HGRP = 3                      # h-tiles per BN1 stats allreduce group
NGRP = KH // HGRP             # 4 staged allreduces
LSL = MLOC // 2               # LIF1 slice width (full tiles)


def _lif1_steps(nc, pool, yv, s1a, ms, md, mlen, tg, nm):
    """LIF chain reading drive slices yv[:, t, ms:ms+mlen] (already
    affined); spikes into s1a[:, t, md:md+mlen] fp16."""
    v = None
    for t in range(T):
        d = yv[:, t, ms : ms + mlen]
        if v is None:
            h = d
        else:
            ht = pool.tile([P, mlen], F32, tag=f"{tg}_h", name=f"{nm}h{t}")
            nc.vector.scalar_tensor_tensor(ht[:], v, 0.5, d, OP.mult, OP.add)
            h = ht[:]
        nc.vector.tensor_scalar(s1a[:, t, md : md + mlen], h, 1.0, None,
                                OP.is_ge)
        if t < T - 1:
            vt = pool.tile([P, mlen], F32, tag=f"{tg}_v", name=f"{nm}v{t}")
            nc.vector.scalar_tensor_tensor(vt[:], h, 1.0, h, OP.is_lt,
                                           OP.mult)
            v = vt[:]


def _bn_coeffs(nc, pool, stg, gt2, bet2, k, pp, tag, dsc_out, dsh_out):
    """From allreduced [pp, 2k] (sum || sumsq) write the fused affine
    coeffs: drive = y*dsc + dsh == 0.5*((y - mean)*rsqrt(var+eps)*g + be).
    gt2/bet2 must be pre-scaled by 0.5."""
    mean = pool.tile([pp, k], F32, tag=f"{tag}_mean", name=f"{tag}_mean")
    nc.vector.tensor_scalar(mean[:], stg[:, 0:k], 1.0 / NTOT, None, OP.mult)
    msq = pool.tile([pp, k], F32, tag=f"{tag}_msq", name=f"{tag}_msq")
    nc.vector.tensor_tensor(msq[:], mean[:], mean[:], OP.mult)
    var = pool.tile([pp, k], F32, tag=f"{tag}_var", name=f"{tag}_var")
    nc.vector.scalar_tensor_tensor(var[:], stg[:, k : 2 * k], 1.0 / NTOT,
                                   msq[:], OP.mult, OP.subtract)
    nc.vector.tensor_scalar(var[:], var[:], EPS, None, OP.add)
    std = pool.tile([pp, k], F32, tag=f"{tag}_std", name=f"{tag}_std")
    nc.scalar.activation(std[:], var[:], AF.Sqrt, bias=0.0, scale=1.0)
    rstd = pool.tile([pp, k], F32, tag=f"{tag}_rstd", name=f"{tag}_rstd")
    nc.vector.reciprocal(rstd[:], std[:])
    nc.vector.tensor_tensor(dsc_out, rstd[:], gt2, OP.mult)
    nc.vector.tensor_tensor(dsh_out, mean[:], dsc_out, OP.mult)
    nc.vector.tensor_tensor(dsh_out, bet2, dsh_out, OP.subtract)


def _build():
    nc = bacc.Bacc("TRN2", target_bir_lowering=False, debug=False,
                   num_devices=NCORES)

    xh_d = nc.dram_tensor("xthi", [KC, P, R], F16, kind="ExternalInput")
    xl_d = nc.dram_tensor("xtlo", [KC, P, R], F16, kind="ExternalInput")
    w1h_d = nc.dram_tensor("w1thi", [KC, P, H], F16, kind="ExternalInput")
    w1l_d = nc.dram_tensor("w1tlo", [KC, P, H], F16, kind="ExternalInput")
    w2h_d = nc.dram_tensor("w2thi", [KH, P, C], F16, kind="ExternalInput")
    w2l_d = nc.dram_tensor("w2tlo", [KH, P, C], F16, kind="ExternalInput")
    g1_d = nc.dram_tensor("g1", [H], F32, kind="ExternalInput")
    be1_d = nc.dram_tensor("be1", [H], F32, kind="ExternalInput")
    g2_d = nc.dram_tensor("g2", [C], F32, kind="ExternalInput")
    be2_d = nc.dram_tensor("be2", [C], F32, kind="ExternalInput")
    out_d = nc.dram_tensor("out", [R, C], F32, kind="ExternalOutput")

    groups = [list(range(NCORES))]

    with tile.TileContext(nc) as tc:
        with (
            tc.tile_pool(name="const", bufs=1) as const,
            tc.tile_pool(name="dram", bufs=1, space="DRAM") as dram,
            tc.tile_pool(name="pw2", bufs=1) as pw2,
        ):
            def colvec(dst_k, src, half=False):
                t_ = const.tile([P, dst_k], F32, name=f"cv_{src.name}",
                                tag=f"cv_{src.name}")
                nc.sync.dma_start(
                    t_[:], src.ap().rearrange("(a p) -> p a", p=P))
                if half:
                    nc.vector.tensor_scalar(t_[:], t_[:], 0.5, None, OP.mult)
                return t_

            def rowvec(src, n, half=False):
                t_ = const.tile([1, n], F32, name=f"rv_{src.name}",
                                tag=f"rv_{src.name}")
                nc.sync.dma_start(t_[:], src.ap().rearrange("(a c) -> a c",
                                                            a=1))
                if half:
                    nc.vector.tensor_scalar(t_[:], t_[:], 0.5, None, OP.mult)
                return t_

            g1t, be1t = colvec(KH, g1_d, True), colvec(KH, be1_d, True)
            g2r, be2r = rowvec(g2_d, C, True), rowvec(be2_d, C, True)

            onesc = const.tile([P, 1], F32)
            nc.vector.memset(onesc[:], 1.0)
            neg1c = const.tile([P, 1], F32)
            nc.vector.memset(neg1c[:], -1.0)
            halfc = const.tile([P, 1], F32)
            nc.vector.memset(halfc[:], 0.5)
            ones1r = const.tile([1, P], F32)
            nc.vector.memset(ones1r[:], 1.0)

            # Warmup collective: absorbs the CC-stream init barrier +
            # launch skew in the shadow of the input DMAs.  Consumed
            # (times zero) in phase B so it can't be DCE'd.
            bar_s = const.tile([1, 8], F32)
            nc.vector.memset(bar_s[:], 1.0)
            bar_in = dram.tile([1, 8], F32, tag="bar_in", name="bar_in")
            bar_out = dram.tile([1, 8], F32, tag="bar_out", name="bar_out")
            nc.gpsimd.dma_start(bar_in[:], bar_s[:])
            nc.gpsimd.collective_compute(
                "AllReduce", OP.add, replica_groups=groups,
                ins=[bar_in.opt()], outs=[bar_out.opt()])

            # persistent across phases
            s1 = [const.tile([P, T, MLOC], F16, tag=f"s1_{k}",
                             name=f"s1_{k}") for k in range(KH)]
            asum1 = const.tile([P, KH * NCH], F32)
            asq1 = const.tile([P, KH * NCH], F32)
            junkA = const.tile([P, NCH], F32)
            dsc1 = const.tile([P, KH], F32)
            dsh1 = const.tile([P, KH], F32)

            y1d = [dram.tile([P, R], F32, tag=f"y1d{a}", name=f"y1d{a}")
                   for a in range(KH)]
            st_in = [dram.tile([P, 2 * HGRP], F32, tag=f"sti{g}",
                               name=f"sti{g}") for g in range(NGRP)]
            st_out = [dram.tile([P, 2 * HGRP], F32, tag=f"sto{g}",
                                name=f"sto{g}") for g in range(NGRP)]
            st2_in = dram.tile([1, 2 * C], F32)
            st2_out = dram.tile([1, 2 * C], F32)

            # ---- phase A: FC1 (h-outer) + staged BN1 stats + LIF1 ------
            with (
                tc.tile_pool(name="pax", bufs=1) as pax,
                tc.tile_pool(name="pbn", bufs=3) as pbn,
                tc.tile_pool(name="prel", bufs=3) as prel,
                tc.tile_pool(name="plif", bufs=2) as plif,
                tc.tile_pool(name="ps_mm", bufs=1, space="PSUM") as ps_mm,
            ):
                w1h = pax.tile([P, KC, H], F16)
                nc.sync.dma_start(w1h[:],
                                  w1h_d.ap().rearrange("k p h -> p k h"))
                xh = pax.tile([P, KC, R], F16)
                xh_src = xh_d.ap().rearrange("k p r -> p k r")
                for k in range(KC):
                    nc.sync.dma_start(xh[:, k : k + 1, :],
                                      xh_src[:, k : k + 1, :])
                w1l = pax.tile([P, KC, H], F16)
                nc.sync.dma_start(w1l[:],
                                  w1l_d.ap().rearrange("k p h -> p k h"))
                xl = pax.tile([P, KC, R], F16)
                nc.sync.dma_start(xl[:], xl_d.ap().rearrange("k p r -> p k r"))
                w2h = pw2.tile([P, KH, C], F16)
                nc.sync.dma_start(w2h[:],
                                  w2h_d.ap().rearrange("k p c -> p k c"))
                w2l = pw2.tile([P, KH, C], F16)
                nc.sync.dma_start(w2l[:],
                                  w2l_d.ap().rearrange("k p c -> p k c"))

                rel = [None] * KH

                def ensure_rel(a_):
                    # reloads ride the DVE queue: paced with the (equally
                    # AR-gated) LIF work, never blocking spills
                    if rel[a_] is None:
                        rt = prel.tile([P, R], F32, tag="rel",
                                       name=f"rel{a_}")
                        nc.gpsimd.dma_start(rt[:], y1d[a_][:])
                        rel[a_] = rt

                def full_lif(a_):
                    ensure_rel(a_)
                    yv = rel[a_][:].rearrange("p (t m) -> p t m", t=T)
                    nc.vector.tensor_scalar(
                        yv[:, :, 0:MLOC], yv[:, :, 0:MLOC],
                        dsc1[:, a_ : a_ + 1],
                        dsh1[:, a_ : a_ + 1], OP.mult, OP.add)
                    for m0 in range(0, MLOC, LSL):
                        _lif1_steps(nc, plif, yv, s1[a_], m0, m0, LSL,
                                    "l1", f"l1_{a_}_{m0}")

                lif_pending = []
                for a in range(KH):
                    pss = [ps_mm.tile([P, CH], F32, tag="mm", bufs=8,
                                      name=f"ps{a}_{c}") for c in range(NCH)]
                    idx = 0
                    for wt, xt in ((w1h, xh), (w1l, xh), (w1h, xl)):
                        for k in range(KC):
                            for c in range(NCH):
                                nc.tensor.matmul(
                                    pss[c][:],
                                    wt[:, k, a * P : (a + 1) * P],
                                    xt[:, k, c * CH : (c + 1) * CH],
                                    start=(idx == 0),
                                    stop=(idx == 8),
                                )
                            idx += 1
                    for c in range(NCH):
                        # evacuate through a bounce ring to DRAM; per-chunk
                        # sum rides the evac via accum_out, sumsq via a
                        # Square written back into the dead psum bank
                        bt = pbn.tile([P, CH], F32, tag="bn",
                                      name=f"bn{a}_{c}")
                        nc.scalar.activation(bt[:], pss[c][:], AF.Identity,
                                             bias=0.0, scale=1.0,
                                             accum_out=asum1[:, a * NCH + c :
                                                             a * NCH + c + 1])
                        nc.scalar.activation(
                            pss[c][:], pss[c][:], AF.Square,
                            bias=0.0, scale=1.0,
                            accum_out=asq1[:, a * NCH + c :
                                           a * NCH + c + 1])
                        nc.sync.dma_start(y1d[a][:, c * CH : (c + 1) * CH],
                                          bt[:])

                    if a % HGRP == HGRP - 1:
                        g = a // HGRP
                        a0 = g * HGRP
                        stg = const.tile([P, 2 * HGRP], F32,
                                         tag=f"stg{g}", name=f"stg{g}")
                        # group-reduce over chunks on the scalar queue
                        # (tiny accum reads) so stat triggers never wait
                        # behind AR-gated DVE work
                        for i in range(HGRP):
                            c0 = (a0 + i) * NCH
                            nc.scalar.activation(
                                junkA[:], asum1[:, c0 : c0 + NCH],
                                AF.Identity, bias=0.0, scale=1.0,
                                accum_out=stg[:, i : i + 1])
                            nc.scalar.activation(
                                junkA[:], asq1[:, c0 : c0 + NCH],
                                AF.Identity, bias=0.0, scale=1.0,
                                accum_out=stg[:, HGRP + i : HGRP + i + 1])
                        nc.gpsimd.dma_start(st_in[g][:], stg[:])
                        nc.gpsimd.collective_compute(
                            "AllReduce", OP.add, replica_groups=groups,
                            ins=[st_in[g].opt()], outs=[st_out[g].opt()])
                        stga = const.tile([P, 2 * HGRP], F32,
                                          tag=f"stga{g}", name=f"stga{g}")
                        nc.gpsimd.dma_start(stga[:], st_out[g][:])
                        _bn_coeffs(nc, const, stga, g1t[:, a0 : a0 + HGRP],
                                   be1t[:, a0 : a0 + HGRP], HGRP, P,
                                   f"bn1g{g}",
                                   dsc1[:, a0 : a0 + HGRP],
                                   dsh1[:, a0 : a0 + HGRP])
                        if g < NGRP - 1:
                            lif_pending.extend(range(a0, a0 + HGRP))

                    # drain one pending LIF h-tile per step (one stats
                    # group behind, so the DVE work hides under FC1)
                    if a >= HGRP and lif_pending:
                        ap_ = lif_pending.pop(0)
                        ensure_rel(ap_)
                        if lif_pending:
                            ensure_rel(lif_pending[0])
                        full_lif(ap_)

                while lif_pending:
                    full_lif(lif_pending.pop(0))

            # ---- phase B: last-group LIF (slice reloads) + FC2 ---------
            with (
                tc.tile_pool(name="pbc", bufs=1) as pbc,
                tc.tile_pool(name="pb", bufs=3) as pb,
                tc.tile_pool(name="prelB", bufs=4) as prelB,
                tc.tile_pool(name="plifB", bufs=2) as plifB,
                tc.tile_pool(name="ps2", bufs=5, space="PSUM") as ps2p,
                tc.tile_pool(name="ps_st", bufs=1, space="PSUM") as ps_st,
            ):
                acc_s = pbc.tile([P, C], F32)
                acc_q = pbc.tile([P, C], F32)
                nc.vector.memset(acc_s[:], 0.0)
                nc.vector.memset(acc_q[:], 0.0)

                a0 = (NGRP - 1) * HGRP    # h-tiles 9..11: sliced here
                y2jb = [pbc.tile([P, T, C], F32, tag=f"y2_{jb}",
                                 name=f"y2_{jb}") for jb in range(NMT + 1)]
                s1f = [s1[k][:].rearrange("p t m -> p (t m)")
                       for k in range(KH)]
                for jb in range(NMT + 1):
                    m0 = jb * MT
                    mlen = MT if jb < NMT else MTAIL
                    # finish s1 for the last stats group on this m-slice,
                    # then immediately emit the FC2 groups that need it
                    for a_ in range(a0, KH):
                        rsl = prelB.tile([P, T, MT], F32, tag="rb",
                                         name=f"rb{a_}_{jb}")
                        src = y1d[a_][:].rearrange("p (t m) -> p t m", t=T)
                        nc.gpsimd.dma_start(rsl[:, :, 0:mlen],
                                            src[:, :, m0 : m0 + mlen])
                        nc.vector.tensor_scalar(
                            rsl[:, :, 0:mlen], rsl[:, :, 0:mlen],
                            dsc1[:, a_ : a_ + 1], dsh1[:, a_ : a_ + 1],
                            OP.mult, OP.add)
                        _lif1_steps(nc, plifB, rsl, s1[a_], 0, m0, mlen,
                                    "lb" if mlen == MT else "lbt",
                                    f"lb{a_}_{jb}")
                    for t in range(T):
                        ps2 = ps2p.tile([P, C], F32, tag="mm2",
                                        name=f"ps2_{jb}_{t}")
                        idx = 0
                        for k in range(KH):
                            for wsp in (w2h, w2l):
                                nc.tensor.matmul(
                                    ps2[:mlen, :],
                                    s1f[k][:, t * MLOC + m0 :
                                           t * MLOC + m0 + mlen],
                                    wsp[:, k, :],
                                    start=(idx == 0),
                                    stop=(idx == 2 * KH - 1),
                                )
                                idx += 1
                        y2t = y2jb[jb]
                        nc.scalar.activation(y2t[:mlen, t, :], ps2[:mlen, :],
                                             AF.Identity, bias=0.0,
                                             scale=1.0)
                        sq2 = pb.tile([P, C], F32, tag="sq2")
                        nc.scalar.activation(sq2[:mlen, :], ps2[:mlen, :],
                                             AF.Square, bias=0.0, scale=1.0)
                        nc.vector.tensor_tensor(acc_s[:mlen, :],
                                                acc_s[:mlen, :],
                                                y2t[:mlen, t, :], OP.add)
                        nc.vector.tensor_tensor(acc_q[:mlen, :],
                                                acc_q[:mlen, :],
                                                sq2[:mlen, :], OP.add)

                # BN2 stats: partition-reduce via ones-matmul, allreduce
                stp_s = ps_st.tile([1, C], F32, tag="sts")
                nc.tensor.matmul(stp_s[:], onesc[:], acc_s[:],
                                 start=True, stop=True)
                stp_q = ps_st.tile([1, C], F32, tag="stq")
                nc.tensor.matmul(stp_q[:], onesc[:], acc_q[:],
                                 start=True, stop=True)
                # consume the warmup-barrier output here (cheap, off the
                # critical path, keeps it from being dead-code-eliminated)
                bar_r = pbc.tile([1, 8], F32)
                nc.gpsimd.dma_start(bar_r[:], bar_out[:])
                st2 = pbc.tile([1, 2 * C], F32)
                nc.vector.tensor_copy(st2[:, 0:C], stp_s[:])
                nc.vector.tensor_copy(st2[:, C : 2 * C], stp_q[:])
                nc.vector.scalar_tensor_tensor(
                    st2[0:1, 0:8], bar_r[:], 0.0, st2[0:1, 0:8],
                    OP.mult, OP.add)
                nc.gpsimd.dma_start(st2_in[:], st2[:])
                nc.gpsimd.collective_compute(
                    "AllReduce", OP.add, replica_groups=groups,
                    ins=[st2_in.opt()], outs=[st2_out.opt()])
                stg2 = pbc.tile([1, 2 * C], F32)
                nc.gpsimd.dma_start(stg2[:], st2_out[:])
                dsc2 = pbc.tile([1, C], F32)
                dsh2 = pbc.tile([1, C], F32)
                _bn_coeffs(nc, pbc, stg2, g2r[:], be2r[:], C, 1, "bn2",
                           dsc2[:], dsh2[:])

                # broadcast [1, C] coeffs to all 128 partitions via PE,
                # replicated along T for batched per-jb affines
                dscB4 = pbc.tile([P, T, C], F32)
                dshB4 = pbc.tile([P, T, C], F32)
                for csrc, dst in ((dsc2, dscB4), (dsh2, dshB4)):
                    bps = ps_st.tile([P, C], F32, tag="bc")
                    nc.tensor.matmul(bps[:], ones1r[:], csrc[:],
                                     start=True, stop=True)
                    for t in range(T):
                        nc.vector.tensor_copy(dst[:, t, :], bps[:])

                # ---- phase C: BN2 affine + LIF2 + direct row-major out --
                for jb in range(NMT + 1):
                    mlen = MT if jb < NMT else MTAIL
                    y2t = y2jb[jb]
                    # batched affine over all T steps of this m-tile
                    nc.vector.tensor_tensor(
                        y2t[:mlen, :, :], y2t[:mlen, :, :], dscB4[:mlen],
                        OP.mult)
                    nc.vector.tensor_tensor(
                        y2t[:mlen, :, :], y2t[:mlen, :, :], dshB4[:mlen],
                        OP.add)
                    v = None
                    for t in range(T):
                        d = y2t[:mlen, t, :]
                        if v is None:
                            h = d
                        else:
                            ht = pb.tile([P, C], F32, tag="l2_h",
                                         name=f"l2h{jb}_{t}")
                            nc.vector.scalar_tensor_tensor(
                                ht[:mlen, :], v, 0.5, d, OP.mult, OP.add)
                            h = ht[:mlen, :]
                        ob = pb.tile([P, C], F32, tag="ob",
                                     name=f"ob{jb}_{t}")
                        if jb < NMT:
                            # spike on the scalar engine: (Sign(h-1)+1)/2
                            # (DVE is the tail bottleneck)
                            sg = pb.tile([P, C], F32, tag="sg",
                                         name=f"sg{jb}_{t}")
                            nc.scalar.activation(sg[:mlen, :], h, AF.Sign,
                                                 bias=neg1c[:mlen, 0:1],
                                                 scale=1.0)
                            nc.scalar.activation(ob[:mlen, :], sg[:mlen, :],
                                                 AF.Identity,
                                                 bias=halfc[:mlen, 0:1],
                                                 scale=halfc[:mlen, 0:1])
                        else:
                            nc.vector.tensor_scalar(ob[:mlen, :], h, 1.0,
                                                    None, OP.is_ge)
                        if t < T - 1:
                            vt = pb.tile([P, C], F32, tag="l2_v",
                                         name=f"l2v{jb}_{t}")
                            nc.vector.scalar_tensor_tensor(
                                vt[:mlen, :], h, 1.0, h, OP.is_lt, OP.mult)
                            v = vt[:mlen, :]
                        r0 = t * MLOC + jb * MT
                        nc.sync.dma_start(out_d[r0 : r0 + mlen, :],
                                          ob[:mlen, :])

    nc.compile()
    return nc


_NC = None
TRACE = False          # set by test harness to capture an NTFF profile
LAST_RESULT = None     # BassKernelResults of the most recent run


def _get_nc():
    global _NC
    if _NC is None:
        _NC = _build()
    return _NC


def _split_f16(a):
    hi = a.astype(np.float16)
    lo = (a - hi.astype(np.float32)).astype(np.float16)
    return np.ascontiguousarray(hi), np.ascontiguousarray(lo)


def _in_maps(x, W1, b1, g1, be1, W2, b2, g2, be2):
    x = np.asarray(x, dtype=np.float32)
    w1t = np.asarray(W1, np.float32).T.reshape(KC, P, H)
    w1thi, w1tlo = _split_f16(w1t)
    w2t = np.asarray(W2, np.float32).T.reshape(KH, P, C)
    w2thi, w2tlo = _split_f16(w2t)
    shared = {
        "w1thi": w1thi, "w1tlo": w1tlo,
        "w2thi": w2thi, "w2tlo": w2tlo,
        "g1": np.asarray(g1, np.float32),
        "be1": np.asarray(be1, np.float32),
        "g2": np.asarray(g2, np.float32),
        "be2": np.asarray(be2, np.float32),
    }
    in_maps = []
    for i in range(NCORES):
        xt = x[:, i * BLOC : (i + 1) * BLOC].reshape(R, C).T.reshape(KC, P, R)
        xthi, xtlo = _split_f16(xt)
        in_maps.append({"xthi": xthi, "xtlo": xtlo, **shared})
    return in_maps


def kernel(x, W1, b1, g1, be1, W2, b2, g2, be2):
    nc = _get_nc()
    in_maps = _in_maps(x, W1, b1, g1, be1, W2, b2, g2, be2)
    res = run_bass_kernel_spmd(nc, in_maps, core_ids=list(range(NCORES)),
                               trace=TRACE)
    global LAST_RESULT
    LAST_RESULT = res
    out = np.concatenate(
        [res.results[i]["out"].reshape(T, BLOC, NN, C) for i in range(NCORES)],
        axis=1,
    )
    return out


# revision 13
# speedup vs baseline: 1.1770x; 1.0355x over previous
"""Trainium2 Bass kernel for the DEER-MLP spiking network (v4).

Network: x(4,32,196,384) -> FC1(384->1536) -> BatchNorm -> LIF(T=4) ->
FC2(1536->384) -> BatchNorm -> LIF -> spikes(4,32,196,384).

Math notes:
 - The reference's 10 DEER Newton iterations over T=4 steps converge to the
   exact sequential LIF recurrence; we compute that directly.
 - The pre-BN biases b1/b2 cancel inside BatchNorm (additive per-channel
   constants are removed by the mean subtraction), so they are dropped.
 - Both matmuls run as multi-pass fp16 with hi/lo fp16 limbs (exact to
   ~2^-22): FC1 = x_hi@w_hi + x_lo@w_hi + x_hi@w_lo; FC2 spikes are exact
   in fp16 so two passes (w_hi + w_lo) suffice.

Distribution: data-parallel over B across 8 cores (784 lanes/core).
BatchNorm statistics are the only cross-core coupling: a warmup dummy
AllReduce (absorbs the runtime's ~50us CC-stream init barrier), four
staged BN1 stat AllReduces (pipelined under FC1), and one BN2 AllReduce.

v4 schedule / queue discipline:
 - FC1 h-tile-outer; y1 spills to DRAM through a bounce ring so FC1 never
   stalls on BN1-stats latency.  Everything stats-critical rides the
   (never-gated) scalar queue: the per-chunk sum via accum_out on the
   Identity evacuation, the sumsq via a Square written back into the dead
   PSUM bank, and the group reductions via tiny accum reads.  The sync
   queue carries only input loads / spills / outputs.
 - All AR-gated work (BN1 affine, LIF1, coeffs, y1 reloads) rides the DVE
   queue, which pays the AR latency anyway; reload DMAs issue there so
   they can never head-of-line block the spill path.
 - The last stats group (h-tiles 9-11) is processed in FC2-m-tile-sized
   slice reloads interleaved with FC2 group emission, so FC2 starts right
   after the final stats AllReduce and never waits again.
 - FC2 uses s1 as the stationary operand, producing y2 ROW-major; w2 is
   prefetched at kernel start into its own pool (no WAR on freed space).
   BN2 stats: DVE accumulate + ones-matmul partition reduce; coeffs
   broadcast to 128 partitions with a rank-1 matmul.  LIF2 emits spikes
   row-major and DMAs straight out - no transpose phase.
"""

import numpy as np

import concourse.bass as bass
import concourse.mybir as mybir
import concourse.tile as tile
from concourse import bacc
from concourse.bass_utils import run_bass_kernel_spmd

F32 = mybir.dt.float32
F16 = mybir.dt.float16
AF = mybir.ActivationFunctionType
OP = mybir.AluOpType
AX = mybir.AxisListType

T, B, NN, C, H = 4, 32, 196, 384, 1536
NCORES = 8
BLOC = B // NCORES            # 4 batches per core
MLOC = BLOC * NN              # 784 lanes per core
R = T * MLOC                  # 3136 flattened (t, m) rows per core
NTOT = T * B * NN             # 25088 batchnorm samples per channel
KC = C // 128                 # 3 c-tiles
KH = H // 128                 # 12 h-tiles
EPS = 1e-5
P = 128

CH = 448                      # FC1 moving-operand chunk (7 * 448 = 3136)
NCH = R // CH
MT = 128                      # FC2 m-tile (rows per output group)
NMT = MLOC // MT              # 6 full m-tiles per t step
MTAIL = MLOC - NMT * MT       # 16 tail rows per t step
HGRP = 3                      # h-tiles per BN1 stats allreduce group
NGRP = KH // HGRP             # 4 staged allreduces
LSL = MLOC // 2               # LIF1 slice width (full tiles)


def _lif1_steps(nc, pool, yv, s1a, ms, md, mlen, tg, nm):
    """LIF chain reading drive slices yv[:, t, ms:ms+mlen] (already
    affined); spikes into s1a[:, t, md:md+mlen] fp16."""
    v = None
    for t in range(T):
        d = yv[:, t, ms : ms + mlen]
        if v is None:
            h = d
        else:
            ht = pool.tile([P, mlen], F32, tag=f"{tg}_h", name=f"{nm}h{t}")
            nc.vector.scalar_tensor_tensor(ht[:], v, 0.5, d, OP.mult, OP.add)
            h = ht[:]
        nc.vector.tensor_scalar(s1a[:, t, md : md + mlen], h, 1.0, None,
                                OP.is_ge)
        if t < T - 1:
            vt = pool.tile([P, mlen], F32, tag=f"{tg}_v", name=f"{nm}v{t}")
            nc.vector.scalar_tensor_tensor(vt[:], h, 1.0, h, OP.is_lt,
                                           OP.mult)
            v = vt[:]


def _bn_coeffs(nc, pool, stg, gt2, bet2, k, pp, tag, dsc_out, dsh_out):
    """From allreduced [pp, 2k] (sum || sumsq) write the fused affine
    coeffs: drive = y*dsc + dsh == 0.5*((y - mean)*rsqrt(var+eps)*g + be).
    gt2/bet2 must be pre-scaled by 0.5."""
    mean = pool.tile([pp, k], F32, tag=f"{tag}_mean", name=f"{tag}_mean")
    nc.vector.tensor_scalar(mean[:], stg[:, 0:k], 1.0 / NTOT, None, OP.mult)
    msq = pool.tile([pp, k], F32, tag=f"{tag}_msq", name=f"{tag}_msq")
    nc.vector.tensor_tensor(msq[:], mean[:], mean[:], OP.mult)
    var = pool.tile([pp, k], F32, tag=f"{tag}_var", name=f"{tag}_var")
    nc.vector.scalar_tensor_tensor(var[:], stg[:, k : 2 * k], 1.0 / NTOT,
                                   msq[:], OP.mult, OP.subtract)
    nc.vector.tensor_scalar(var[:], var[:], EPS, None, OP.add)
    std = pool.tile([pp, k], F32, tag=f"{tag}_std", name=f"{tag}_std")
    nc.scalar.activation(std[:], var[:], AF.Sqrt, bias=0.0, scale=1.0)
    rstd = pool.tile([pp, k], F32, tag=f"{tag}_rstd", name=f"{tag}_rstd")
    nc.vector.reciprocal(rstd[:], std[:])
    nc.vector.tensor_tensor(dsc_out, rstd[:], gt2, OP.mult)
    nc.vector.tensor_tensor(dsh_out, mean[:], dsc_out, OP.mult)
    nc.vector.tensor_tensor(dsh_out, bet2, dsh_out, OP.subtract)


def _build():
    nc = bacc.Bacc("TRN2", target_bir_lowering=False, debug=False,
                   num_devices=NCORES)

    xh_d = nc.dram_tensor("xthi", [KC, P, R], F16, kind="ExternalInput")
    xl_d = nc.dram_tensor("xtlo", [KC, P, R], F16, kind="ExternalInput")
    w1h_d = nc.dram_tensor("w1thi", [KC, P, H], F16, kind="ExternalInput")
    w1l_d = nc.dram_tensor("w1tlo", [KC, P, H], F16, kind="ExternalInput")
    w2h_d = nc.dram_tensor("w2thi", [KH, P, C], F16, kind="ExternalInput")
    w2l_d = nc.dram_tensor("w2tlo", [KH, P, C], F16, kind="ExternalInput")
    g1_d = nc.dram_tensor("g1", [H], F32, kind="ExternalInput")
    be1_d = nc.dram_tensor("be1", [H], F32, kind="ExternalInput")
    g2_d = nc.dram_tensor("g2", [C], F32, kind="ExternalInput")
    be2_d = nc.dram_tensor("be2", [C], F32, kind="ExternalInput")
    out_d = nc.dram_tensor("out", [R, C], F32, kind="ExternalOutput")

    groups = [list(range(NCORES))]

    with tile.TileContext(nc) as tc:
        with (
            tc.tile_pool(name="const", bufs=1) as const,
            tc.tile_pool(name="dram", bufs=1, space="DRAM") as dram,
            tc.tile_pool(name="pw2", bufs=1) as pw2,
        ):
            def colvec(dst_k, src, half=False):
                t_ = const.tile([P, dst_k], F32, name=f"cv_{src.name}",
                                tag=f"cv_{src.name}")
                nc.sync.dma_start(
                    t_[:], src.ap().rearrange("(a p) -> p a", p=P))
                if half:
                    nc.vector.tensor_scalar(t_[:], t_[:], 0.5, None, OP.mult)
                return t_

            def rowvec(src, n, half=False):
                t_ = const.tile([1, n], F32, name=f"rv_{src.name}",
                                tag=f"rv_{src.name}")
                nc.sync.dma_start(t_[:], src.ap().rearrange("(a c) -> a c",
                                                            a=1))
                if half:
                    nc.vector.tensor_scalar(t_[:], t_[:], 0.5, None, OP.mult)
                return t_

            g1t, be1t = colvec(KH, g1_d, True), colvec(KH, be1_d, True)
            g2r, be2r = rowvec(g2_d, C, True), rowvec(be2_d, C, True)

            onesc = const.tile([P, 1], F32)
            nc.vector.memset(onesc[:], 1.0)
            neg1c = const.tile([P, 1], F32)
            nc.vector.memset(neg1c[:], -1.0)
            halfc = const.tile([P, 1], F32)
            nc.vector.memset(halfc[:], 0.5)
            ones1r = const.tile([1, P], F32)
            nc.vector.memset(ones1r[:], 1.0)

            # Warmup collective: absorbs the CC-stream init barrier +
            # launch skew in the shadow of the input DMAs.  Consumed
            # (times zero) in phase B so it can't be DCE'd.
            bar_s = const.tile([1, 8], F32)
            nc.vector.memset(bar_s[:], 1.0)
            bar_in = dram.tile([1, 8], F32, tag="bar_in", name="bar_in")
            bar_out = dram.tile([1, 8], F32, tag="bar_out", name="bar_out")
            nc.gpsimd.dma_start(bar_in[:], bar_s[:])
            nc.gpsimd.collective_compute(
                "AllReduce", OP.add, replica_groups=groups,
                ins=[bar_in.opt()], outs=[bar_out.opt()])

            # persistent across phases
            s1 = [const.tile([P, T, MLOC], F16, tag=f"s1_{k}",
                             name=f"s1_{k}") for k in range(KH)]
            asum1 = const.tile([P, KH * NCH], F32)
            asq1 = const.tile([P, KH * NCH], F32)
            junkA = const.tile([P, NCH], F32)
            dsc1 = const.tile([P, KH], F32)
            dsh1 = const.tile([P, KH], F32)

            y1d = [dram.tile([P, R], F32, tag=f"y1d{a}", name=f"y1d{a}")
                   for a in range(KH)]
            st_in = [dram.tile([P, 2 * HGRP], F32, tag=f"sti{g}",
                               name=f"sti{g}") for g in range(NGRP)]
            st_out = [dram.tile([P, 2 * HGRP], F32, tag=f"sto{g}",
                                name=f"sto{g}") for g in range(NGRP)]
            st2_in = dram.tile([1, 2 * C], F32)
            st2_out = dram.tile([1, 2 * C], F32)

            # ---- phase A: FC1 (h-outer) + staged BN1 stats + LIF1 ------
            with (
                tc.tile_pool(name="pax", bufs=1) as pax,
                tc.tile_pool(name="pbn", bufs=3) as pbn,
                tc.tile_pool(name="prel", bufs=3) as prel,
                tc.tile_pool(name="plif", bufs=2) as plif,
                tc.tile_pool(name="ps_mm", bufs=1, space="PSUM") as ps_mm,
            ):
                w1h = pax.tile([P, KC, H], F16)
                nc.sync.dma_start(w1h[:],
                                  w1h_d.ap().rearrange("k p h -> p k h"))
                xh = pax.tile([P, KC, R], F16)
                xh_src = xh_d.ap().rearrange("k p r -> p k r")
                for k in range(KC):
                    nc.sync.dma_start(xh[:, k : k + 1, :],
                                      xh_src[:, k : k + 1, :])
                w1l = pax.tile([P, KC, H], F16)
                nc.sync.dma_start(w1l[:],
                                  w1l_d.ap().rearrange("k p h -> p k h"))
                xl = pax.tile([P, KC, R], F16)
                nc.sync.dma_start(xl[:], xl_d.ap().rearrange("k p r -> p k r"))
                w2h = pw2.tile([P, KH, C], F16)
                nc.sync.dma_start(w2h[:],
                                  w2h_d.ap().rearrange("k p c -> p k c"))
                w2l = pw2.tile([P, KH, C], F16)
                nc.sync.dma_start(w2l[:],
                                  w2l_d.ap().rearrange("k p c -> p k c"))

                rel = [None] * KH

                def ensure_rel(a_):
                    # reloads ride the DVE queue: paced with the (equally
                    # AR-gated) LIF work, never blocking spills
                    if rel[a_] is None:
                        rt = prel.tile([P, R], F32, tag="rel",
                                       name=f"rel{a_}")
                        nc.gpsimd.dma_start(rt[:], y1d[a_][:])
                        rel[a_] = rt

                def full_lif(a_):
                    ensure_rel(a_)
                    yv = rel[a_][:].rearrange("p (t m) -> p t m", t=T)
                    nc.vector.tensor_scalar(
                        yv[:, :, 0:MLOC], yv[:, :, 0:MLOC],
                        dsc1[:, a_ : a_ + 1],
                        dsh1[:, a_ : a_ + 1], OP.mult, OP.add)
                    for m0 in range(0, MLOC, LSL):
                        _lif1_steps(nc, plif, yv, s1[a_], m0, m0, LSL,
                                    "l1", f"l1_{a_}_{m0}")

                lif_pending = []
                for a in range(KH):
                    for c in range(NCH):
                        ps = ps_mm.tile([P, CH], F32, tag="mm", bufs=8,
                                        name=f"ps{a}_{c}")
                        idx = 0
                        for wt, xt in ((w1h, xh), (w1l, xh), (w1h, xl)):
                            for k in range(KC):
                                nc.tensor.matmul(
                                    ps[:],
                                    wt[:, k, a * P : (a + 1) * P],
                                    xt[:, k, c * CH : (c + 1) * CH],
                                    start=(idx == 0),
                                    stop=(idx == 8),
                                )
                                idx += 1
                        # evacuate through a bounce ring to DRAM; per-chunk
                        # sum rides the evac via accum_out, sumsq via a
                        # Square written back into the dead psum bank
                        bt = pbn.tile([P, CH], F32, tag="bn",
                                      name=f"bn{a}_{c}")
                        nc.scalar.activation(bt[:], ps[:], AF.Identity,
                                             bias=0.0, scale=1.0,
                                             accum_out=asum1[:, a * NCH + c :
                                                             a * NCH + c + 1])
                        nc.scalar.activation(
                            ps[:], ps[:], AF.Square,
                            bias=0.0, scale=1.0,
                            accum_out=asq1[:, a * NCH + c :
                                           a * NCH + c + 1])
                        nc.sync.dma_start(y1d[a][:, c * CH : (c + 1) * CH],
                                          bt[:])

                    if a % HGRP == HGRP - 1:
                        g = a // HGRP
                        a0 = g * HGRP
                        stg = const.tile([P, 2 * HGRP], F32,
                                         tag=f"stg{g}", name=f"stg{g}")
                        # group-reduce over chunks on the scalar queue
                        # (tiny accum reads) so stat triggers never wait
                        # behind AR-gated DVE work
                        for i in range(HGRP):
                            c0 = (a0 + i) * NCH
                            nc.scalar.activation(
                                junkA[:], asum1[:, c0 : c0 + NCH],
                                AF.Identity, bias=0.0, scale=1.0,
                                accum_out=stg[:, i : i + 1])
                            nc.scalar.activation(
                                junkA[:], asq1[:, c0 : c0 + NCH],
                                AF.Identity, bias=0.0, scale=1.0,
                                accum_out=stg[:, HGRP + i : HGRP + i + 1])
                        nc.gpsimd.dma_start(st_in[g][:], stg[:])
                        nc.gpsimd.collective_compute(
                            "AllReduce", OP.add, replica_groups=groups,
                            ins=[st_in[g].opt()], outs=[st_out[g].opt()])
                        stga = const.tile([P, 2 * HGRP], F32,
                                          tag=f"stga{g}", name=f"stga{g}")
                        nc.gpsimd.dma_start(stga[:], st_out[g][:])
                        _bn_coeffs(nc, const, stga, g1t[:, a0 : a0 + HGRP],
                                   be1t[:, a0 : a0 + HGRP], HGRP, P,
                                   f"bn1g{g}",
                                   dsc1[:, a0 : a0 + HGRP],
                                   dsh1[:, a0 : a0 + HGRP])
                        if g < NGRP - 1:
                            lif_pending.extend(range(a0, a0 + HGRP))

                    # drain one pending LIF h-tile per step (one stats
                    # group behind, so the DVE work hides under FC1)
                    if a >= HGRP and lif_pending:
                        ap_ = lif_pending.pop(0)
                        ensure_rel(ap_)
                        if lif_pending:
                            ensure_rel(lif_pending[0])
                        full_lif(ap_)

                while lif_pending:
                    full_lif(lif_pending.pop(0))

            # ---- phase B: last-group LIF (slice reloads) + FC2 ---------
            with (
                tc.tile_pool(name="pbc", bufs=1) as pbc,
                tc.tile_pool(name="pb", bufs=3) as pb,
                tc.tile_pool(name="prelB", bufs=4) as prelB,
                tc.tile_pool(name="plifB", bufs=2) as plifB,
                tc.tile_pool(name="ps2", bufs=5, space="PSUM") as ps2p,
                tc.tile_pool(name="ps_st", bufs=1, space="PSUM") as ps_st,
            ):
                acc_s = pbc.tile([P, C], F32)
                acc_q = pbc.tile([P, C], F32)
                nc.vector.memset(acc_s[:], 0.0)
                nc.vector.memset(acc_q[:], 0.0)

                a0 = (NGRP - 1) * HGRP    # h-tiles 9..11: sliced here
                y2jb = [pbc.tile([P, T, C], F32, tag=f"y2_{jb}",
                                 name=f"y2_{jb}") for jb in range(NMT + 1)]
                s1f = [s1[k][:].rearrange("p t m -> p (t m)")
                       for k in range(KH)]
                for jb in range(NMT + 1):
                    m0 = jb * MT
                    mlen = MT if jb < NMT else MTAIL
                    # finish s1 for the last stats group on this m-slice,
                    # then immediately emit the FC2 groups that need it
                    for a_ in range(a0, KH):
                        rsl = prelB.tile([P, T, MT], F32, tag="rb",
                                         name=f"rb{a_}_{jb}")
                        src = y1d[a_][:].rearrange("p (t m) -> p t m", t=T)
                        nc.gpsimd.dma_start(rsl[:, :, 0:mlen],
                                            src[:, :, m0 : m0 + mlen])
                        nc.scalar.activation(
                            rsl[:, :, 0:mlen], rsl[:, :, 0:mlen],
                            AF.Identity, bias=dsh1[:, a_ : a_ + 1],
                            scale=dsc1[:, a_ : a_ + 1])
                        _lif1_steps(nc, plifB, rsl, s1[a_], 0, m0, mlen,
                                    "lb" if mlen == MT else "lbt",
                                    f"lb{a_}_{jb}")
                    for t in range(T):
                        ps2 = ps2p.tile([P, C], F32, tag="mm2",
                                        name=f"ps2_{jb}_{t}")
                        idx = 0
                        for k in range(KH):
                            for wsp in (w2h, w2l):
                                nc.tensor.matmul(
                                    ps2[:mlen, :],
                                    s1f[k][:, t * MLOC + m0 :
                                           t * MLOC + m0 + mlen],
                                    wsp[:, k, :],
                                    start=(idx == 0),
                                    stop=(idx == 2 * KH - 1),
                                )
                                idx += 1
                        y2t = y2jb[jb]
                        nc.scalar.activation(y2t[:mlen, t, :], ps2[:mlen, :],
                                             AF.Identity, bias=0.0,
                                             scale=1.0)
                        sq2 = pb.tile([P, C], F32, tag="sq2")
                        nc.scalar.activation(sq2[:mlen, :], ps2[:mlen, :],
                                             AF.Square, bias=0.0, scale=1.0)
                        nc.vector.tensor_tensor(acc_s[:mlen, :],
                                                acc_s[:mlen, :],
                                                y2t[:mlen, t, :], OP.add)
                        nc.vector.tensor_tensor(acc_q[:mlen, :],
                                                acc_q[:mlen, :],
                                                sq2[:mlen, :], OP.add)

                # BN2 stats: partition-reduce via ones-matmul, allreduce
                stp_s = ps_st.tile([1, C], F32, tag="sts")
                nc.tensor.matmul(stp_s[:], onesc[:], acc_s[:],
                                 start=True, stop=True)
                stp_q = ps_st.tile([1, C], F32, tag="stq")
                nc.tensor.matmul(stp_q[:], onesc[:], acc_q[:],
                                 start=True, stop=True)
                # consume the warmup-barrier output here (cheap, off the
                # critical path, keeps it from being dead-code-eliminated)
                bar_r = pbc.tile([1, 8], F32)
                nc.gpsimd.dma_start(bar_r[:], bar_out[:])
                st2 = pbc.tile([1, 2 * C], F32)
                nc.vector.tensor_copy(st2[:, 0:C], stp_s[:])
                nc.vector.tensor_copy(st2[:, C : 2 * C], stp_q[:])
                nc.vector.scalar_tensor_tensor(
                    st2[0:1, 0:8], bar_r[:], 0.0, st2[0:1, 0:8],
                    OP.mult, OP.add)
                nc.gpsimd.dma_start(st2_in[:], st2[:])
                nc.gpsimd.collective_compute(
                    "AllReduce", OP.add, replica_groups=groups,
                    ins=[st2_in.opt()], outs=[st2_out.opt()])
                stg2 = pbc.tile([1, 2 * C], F32)
                nc.gpsimd.dma_start(stg2[:], st2_out[:])
                dsc2 = pbc.tile([1, C], F32)
                dsh2 = pbc.tile([1, C], F32)
                _bn_coeffs(nc, pbc, stg2, g2r[:], be2r[:], C, 1, "bn2",
                           dsc2[:], dsh2[:])

                # broadcast [1, C] coeffs to all 128 partitions via PE,
                # replicated along T for batched per-jb affines
                dscB4 = pbc.tile([P, T, C], F32)
                dshB4 = pbc.tile([P, T, C], F32)
                for csrc, dst in ((dsc2, dscB4), (dsh2, dshB4)):
                    bps = ps_st.tile([P, C], F32, tag="bc")
                    nc.tensor.matmul(bps[:], ones1r[:], csrc[:],
                                     start=True, stop=True)
                    for t in range(T):
                        nc.vector.tensor_copy(dst[:, t, :], bps[:])

                # ---- phase C: BN2 affine + LIF2 + direct row-major out --
                for jb in range(NMT + 1):
                    mlen = MT if jb < NMT else MTAIL
                    y2t = y2jb[jb]
                    # batched affine over all T steps of this m-tile
                    nc.vector.tensor_tensor(
                        y2t[:mlen, :, :], y2t[:mlen, :, :], dscB4[:mlen],
                        OP.mult)
                    nc.vector.tensor_tensor(
                        y2t[:mlen, :, :], y2t[:mlen, :, :], dshB4[:mlen],
                        OP.add)
                    v = None
                    for t in range(T):
                        d = y2t[:mlen, t, :]
                        if v is None:
                            h = d
                        else:
                            ht = pb.tile([P, C], F32, tag="l2_h",
                                         name=f"l2h{jb}_{t}")
                            nc.vector.scalar_tensor_tensor(
                                ht[:mlen, :], v, 0.5, d, OP.mult, OP.add)
                            h = ht[:mlen, :]
                        ob = pb.tile([P, C], F32, tag="ob",
                                     name=f"ob{jb}_{t}")
                        if jb < NMT:
                            # spike on the scalar engine: (Sign(h-1)+1)/2
                            # (DVE is the tail bottleneck)
                            sg = pb.tile([P, C], F32, tag="sg",
                                         name=f"sg{jb}_{t}")
                            nc.scalar.activation(sg[:mlen, :], h, AF.Sign,
                                                 bias=neg1c[:mlen, 0:1],
                                                 scale=1.0)
                            nc.scalar.activation(ob[:mlen, :], sg[:mlen, :],
                                                 AF.Identity,
                                                 bias=halfc[:mlen, 0:1],
                                                 scale=halfc[:mlen, 0:1])
                        else:
                            nc.vector.tensor_scalar(ob[:mlen, :], h, 1.0,
                                                    None, OP.is_ge)
                        if t < T - 1:
                            vt = pb.tile([P, C], F32, tag="l2_v",
                                         name=f"l2v{jb}_{t}")
                            nc.vector.scalar_tensor_tensor(
                                vt[:mlen, :], h, 1.0, h, OP.is_lt, OP.mult)
                            v = vt[:mlen, :]
                        r0 = t * MLOC + jb * MT
                        nc.sync.dma_start(out_d[r0 : r0 + mlen, :],
                                          ob[:mlen, :])

    nc.compile()
    return nc


_NC = None
TRACE = False          # set by test harness to capture an NTFF profile
LAST_RESULT = None     # BassKernelResults of the most recent run


def _get_nc():
    global _NC
    if _NC is None:
        _NC = _build()
    return _NC


def _split_f16(a):
    hi = a.astype(np.float16)
    lo = (a - hi.astype(np.float32)).astype(np.float16)
    return np.ascontiguousarray(hi), np.ascontiguousarray(lo)


def _in_maps(x, W1, b1, g1, be1, W2, b2, g2, be2):
    x = np.asarray(x, dtype=np.float32)
    w1t = np.asarray(W1, np.float32).T.reshape(KC, P, H)
    w1thi, w1tlo = _split_f16(w1t)
    w2t = np.asarray(W2, np.float32).T.reshape(KH, P, C)
    w2thi, w2tlo = _split_f16(w2t)
    shared = {
        "w1thi": w1thi, "w1tlo": w1tlo,
        "w2thi": w2thi, "w2tlo": w2tlo,
        "g1": np.asarray(g1, np.float32),
        "be1": np.asarray(be1, np.float32),
        "g2": np.asarray(g2, np.float32),
        "be2": np.asarray(be2, np.float32),
    }
    in_maps = []
    for i in range(NCORES):
        xt = x[:, i * BLOC : (i + 1) * BLOC].reshape(R, C).T.reshape(KC, P, R)
        xthi, xtlo = _split_f16(xt)
        in_maps.append({"xthi": xthi, "xtlo": xtlo, **shared})
    return in_maps


def kernel(x, W1, b1, g1, be1, W2, b2, g2, be2):
    nc = _get_nc()
    in_maps = _in_maps(x, W1, b1, g1, be1, W2, b2, g2, be2)
    res = run_bass_kernel_spmd(nc, in_maps, core_ids=list(range(NCORES)),
                               trace=TRACE)
    global LAST_RESULT
    LAST_RESULT = res
    out = np.concatenate(
        [res.results[i]["out"].reshape(T, BLOC, NN, C) for i in range(NCORES)],
        axis=1,
    )
    return out
